# revision 1
# baseline (speedup 1.0000x reference)
"""GroupedQueryAttention (B=1, T=2048, C=2048, H=16, KVH=4, D=128) on 8 trn2 cores.

Sharding: tensor-parallel over heads. Core c owns q-heads {2c, 2c+1} and kv-head
c//2 (both q-heads of a core map to the same kv head since group size is 4).
Wq/Wk/Wv are column-sliced and Wo row-sliced on the host; each core computes a
full o_proj partial [2048, 2048] and the host sums the 8 partials (the
all-reduce after o_proj).

Per-core pipeline (all matmuls in float32r = full-rate fp32 on the PE):
  1. x row-tile [128, 2048] -> PE-transpose -> x^T c-tiles (lhsT for projections)
  2. qkv = x @ [Wq_c | Wk_c | Wv_c]  -> [128 t, 512] PSUM per t-tile
  3. per-head LayerNorm in [t, d] layout (free-dim reduces; apply via ACT with
     per-partition scale/bias); softmax scale * gq * gk folded into k^T rows
  4. PE-transpose q,k -> q^T [d, t], k^T [d, s]; v kept as [s, d] (attn lhsT)
  5. causal attention per head, t-superblocks of 512: S = q^T.T @ k^T chunks,
     fused mask+max (tensor_tensor_reduce), Exp with fused row-sum (accum_out),
     P^T via PE-transpose against diag(1/rowsum) (free normalization),
     attn@v -> headout^T [d, t] (exactly o_proj's lhsT layout)
  6. o_partial = headout_c @ Wo_c -> DRAM
"""

from contextlib import ExitStack

import numpy as np

import concourse.bass as bass
import concourse.bacc as bacc
import concourse.tile as tile
from concourse import mybir
from concourse import bass_utils

P = 128
T = 2048
C = 2048
NT = T // P  # 16 t-tiles
SB = 512  # superblock width
NSB = T // SB  # 4 superblocks
F32 = mybir.dt.float32
F32R = mybir.dt.float32r
AF = mybir.ActivationFunctionType
ALU = mybir.AluOpType
NEG_BIG = -1.0e30

N_CORES = 8


def _build(mm_dt=F32R, tr_dt=F32, stage=3, safe=1):
    nc = bacc.Bacc("TRN2", target_bir_lowering=False, debug=False,
                   num_devices=N_CORES)
    x_d = nc.dram_tensor("x", [T, C], F32, kind="ExternalInput").ap()
    wqkv_d = nc.dram_tensor("wqkv", [C, 512], F32, kind="ExternalInput").ap()
    wo_d = nc.dram_tensor("wo", [256, C], F32, kind="ExternalInput").ap()
    fk_d = nc.dram_tensor("fk", [P, 1], F32, kind="ExternalInput").ap()
    masks_d = nc.dram_tensor("masks", [P, 5 * SB], F32, kind="ExternalInput").ap()
    ident_d = nc.dram_tensor("ident", [P, P], F32, kind="ExternalInput").ap()
    out_d = nc.dram_tensor("out", [T, C], F32, kind="ExternalOutput").ap()

    def mm(a):
        return a.bitcast(mm_dt)

    def tr(a):
        return a.bitcast(tr_dt)

    with tile.TileContext(nc) as tc, ExitStack() as ctx:
        const = ctx.enter_context(tc.tile_pool(name="const", bufs=1))
        persist = ctx.enter_context(tc.tile_pool(name="persist", bufs=1))
        stats = ctx.enter_context(tc.tile_pool(name="stats", bufs=24))

        ident = const.tile([P, P], F32, tag="ident")
        nc.sync.dma_start(ident[:], ident_d)
        fk = const.tile([P, 1], F32, tag="fk")
        nc.sync.dma_start(fk[:], fk_d)
        masks = const.tile([P, 5 * SB], F32, tag="masks")
        nc.sync.dma_start(masks[:], masks_d)
        wo = const.tile([P, 2 * C], mm_dt, tag="wo")
        for kk in range(2):
            nc.sync.dma_start(wo[:, kk * C:(kk + 1) * C],
                              wo_d[kk * P:(kk + 1) * P, :].bitcast(mm_dt))

        epsb = const.tile([P, 1], F32, tag="epsb")
        nc.vector.memset(epsb[:], 1e-5)
        qT = persist.tile([P, 2 * T], mm_dt, tag="qT")   # [d, t] per head
        kT = persist.tile([P, T], mm_dt, tag="kT")       # [d, s]
        vv = persist.tile([P, T], mm_dt, tag="vv")       # s-tile j at cols j*128
        hoT = persist.tile([P, 2 * T], mm_dt, tag="hoT")  # [d, t] per head

        # ---------------- Phase 1: projections + LN + transposes ----------
        with tc.tile_pool(name="p1", bufs=1) as p1, \
             tc.tile_pool(name="xrow", bufs=2) as xrow_p, \
             tc.tile_pool(name="xT", bufs=2) as xt_p, \
             tc.tile_pool(name="qln", bufs=2) as qln_p, \
             tc.tile_pool(name="psA", bufs=4, space="PSUM") as psA, \
             tc.tile_pool(name="psB", bufs=2, space="PSUM") as psB:
            wq = p1.tile([P, NT * 512], mm_dt, tag="wqkv")
            for k in range(NT):
                nc.sync.dma_start(wq[:, k * 512:(k + 1) * 512],
                                  wqkv_d[k * P:(k + 1) * P, :].bitcast(mm_dt))

            for i in range(NT):
                xr = xrow_p.tile([P, C], F32, tag="xr")
                nc.sync.dma_start(xr[:], x_d[i * P:(i + 1) * P, :])
                xT = xt_p.tile([P, C], mm_dt, tag="xT")
                for k in range(NT):
                    pt = psA.tile([P, P], F32, tag="tp")
                    nc.tensor.transpose(pt[:], tr(xr[:, k * P:(k + 1) * P]),
                                        tr(ident[:]))
                    nc.any.tensor_copy(xT[:, k * P:(k + 1) * P], pt[:])
                qk = psB.tile([P, 512], F32, tag="qkv")
                for k in range(NT):
                    nc.tensor.matmul(qk[:], mm(xT[:, k * P:(k + 1) * P]),
                                     mm(wq[:, k * 512:(k + 1) * 512]),
                                     start=(k == 0), stop=(k == NT - 1))
                # LayerNorm for q0, q1, k (cols 0:128, 128:256, 256:384)
                qln = qln_p.tile([P, 384], F32, tag="qln")
                for ci in range(3):
                    col = qk[:, ci * P:(ci + 1) * P]
                    sq = qln_p.tile([P, P], F32, tag="sq")
                    nc.scalar.square(sq[:], col)
                    s1 = stats.tile([P, 1], F32, tag="st")
                    nc.vector.reduce_sum(s1[:], col, mybir.AxisListType.X)
                    s2 = stats.tile([P, 1], F32, tag="st")
                    nc.vector.reduce_sum(s2[:], sq[:], mybir.AxisListType.X)
                    mu = stats.tile([P, 1], F32, tag="st")
                    nc.vector.tensor_scalar_mul(mu[:], s1[:], 1.0 / P)
                    ex2 = stats.tile([P, 1], F32, tag="st")
                    nc.vector.tensor_scalar_mul(ex2[:], s2[:], 1.0 / P)
                    msq = stats.tile([P, 1], F32, tag="st")
                    nc.scalar.square(msq[:], mu[:])
                    var = stats.tile([P, 1], F32, tag="st")
                    nc.vector.scalar_tensor_tensor(
                        var[:], ex2[:], 1.0, msq[:], ALU.mult, ALU.subtract)
                    sd = stats.tile([P, 1], F32, tag="st")
                    nc.scalar.activation(sd[:], var[:], AF.Sqrt, bias=epsb[:])
                    rs = stats.tile([P, 1], F32, tag="st")
                    nc.vector.reciprocal(rs[:], sd[:])
                    nb = stats.tile([P, 1], F32, tag="st")
                    nc.vector.scalar_tensor_tensor(
                        nb[:], mu[:], -1.0, rs[:], ALU.mult, ALU.mult)
                    nc.scalar.activation(qln[:, ci * P:(ci + 1) * P], col,
                                         AF.Identity, bias=nb[:], scale=rs[:])
                for ci in range(3):
                    pt = psA.tile([P, P], F32, tag="tp")
                    nc.tensor.transpose(pt[:], tr(qln[:, ci * P:(ci + 1) * P]),
                                        tr(ident[:]))
                    if ci < 2:
                        nc.any.tensor_copy(qT[:, ci * T + i * P:ci * T + (i + 1) * P],
                                           pt[:])
                    else:
                        nc.vector.tensor_scalar_mul(kT[:, i * P:(i + 1) * P],
                                                    pt[:], fk[:])
                nc.any.tensor_copy(vv[:, i * P:(i + 1) * P], qk[:, 384:512])

        # ---------------- Phase 2: causal attention per head --------------
        if stage >= 2:
         with tc.tile_pool(name="sbuf_s", bufs=1) as s_pool, \
             tc.tile_pool(name="ptb", bufs=3) as pt_pool, \
             tc.tile_pool(name="dg", bufs=2) as dg_pool, \
             tc.tile_pool(name="psS", bufs=4, space="PSUM") as psS, \
             tc.tile_pool(name="psPT", bufs=2, space="PSUM") as psPT, \
             tc.tile_pool(name="psO", bufs=2, space="PSUM") as psO:
            for I in range(NSB):
                L = (I + 1) * SB
                for h in range(2):
                    S = s_pool.tile([P, 4 * T], F32, tag="S")
                    m = [None] * 4
                    for J in range(I + 1):
                        for p in range(4):
                            sp = psS.tile([P, SB], F32, tag="sp")
                            lq = qT[:, h * T + (I * 4 + p) * P:
                                    h * T + (I * 4 + p + 1) * P]
                            nc.tensor.matmul(sp[:], mm(lq),
                                             mm(kT[:, J * SB:(J + 1) * SB]),
                                             start=True, stop=True)
                            msl = (masks[:, p * SB:(p + 1) * SB] if J == I
                                   else masks[:, 4 * SB:5 * SB])
                            mnew = stats.tile([P, 1], F32, tag="st")
                            if not (safe & 1):
                                nc.vector.tensor_tensor_reduce(
                                    out=S[:, p * T + J * SB:p * T + (J + 1) * SB],
                                    in0=sp[:], in1=msl, scale=1.0,
                                    scalar=(m[p][:] if m[p] is not None else -3.0e38),
                                    op0=ALU.add, op1=ALU.max, accum_out=mnew[:])
                            else:
                                dst = S[:, p * T + J * SB:p * T + (J + 1) * SB]
                                nc.vector.scalar_tensor_tensor(
                                    dst, sp[:], 1.0, msl, ALU.mult, ALU.add)
                                cmax = stats.tile([P, 1], F32, tag="st")
                                nc.vector.reduce_max(cmax[:], dst,
                                                     mybir.AxisListType.X)
                                if m[p] is None:
                                    mnew = cmax
                                else:
                                    nc.vector.scalar_tensor_tensor(
                                        mnew[:], cmax[:], 1.0, m[p][:],
                                        ALU.mult, ALU.max)
                            m[p] = mnew
                    Pb = None
                    if safe & 4:
                        Pb = s_pool.tile([P, 4 * T], F32, tag="Pb")
                    for p in range(4):
                        nm = stats.tile([P, 1], F32, tag="st")
                        nc.scalar.mul(nm[:], m[p][:], -1.0)
                        sm = stats.tile([P, 1], F32, tag="st")
                        esc = dg_pool.tile([P, 4 * SB], F32, tag="esc")
                        if not (safe & 2):
                            nc.scalar.activation(esc[:, 0:L],
                                                 S[:, p * T:p * T + L],
                                                 AF.Exp, bias=nm[:],
                                                 accum_out=sm[:])
                        else:
                            nc.scalar.activation(esc[:, 0:L],
                                                 S[:, p * T:p * T + L],
                                                 AF.Exp, bias=nm[:])
                            nc.vector.reduce_sum(sm[:], esc[:, 0:L],
                                                 mybir.AxisListType.X)
                        lz = stats.tile([P, 1], F32, tag="st")
                        nc.scalar.activation(lz[:], sm[:], AF.Ln)
                        b2 = stats.tile([P, 1], F32, tag="st")
                        nc.vector.scalar_tensor_tensor(
                            b2[:], m[p][:], -1.0, lz[:], ALU.mult, ALU.subtract)
                        if not (safe & 4):
                            nc.scalar.activation(S[:, p * T:p * T + L],
                                                 S[:, p * T:p * T + L],
                                                 AF.Exp, bias=b2[:])
                        else:
                            nc.scalar.activation(Pb[:, p * T:p * T + L],
                                                 S[:, p * T:p * T + L],
                                                 AF.Exp, bias=b2[:])
                            if p == 3:
                                S = Pb
                    oT = psO.tile([P, SB], F32, tag="oT")
                    nst = 4 * (I + 1)
                    for j in range(nst):
                        ptp = psPT.tile([P, SB], F32, tag="ptp")
                        for p in range(4):
                            nc.tensor.transpose(
                                ptp[:, p * P:(p + 1) * P],
                                tr(S[:, p * T + j * P:p * T + (j + 1) * P]),
                                tr(ident[:]))
                        pts = pt_pool.tile([P, SB], mm_dt, tag="pts")
                        nc.any.tensor_copy(pts[:], ptp[:])
                        nc.tensor.matmul(oT[:], mm(vv[:, j * P:(j + 1) * P]),
                                         mm(pts[:]),
                                         start=(j == 0), stop=(j == nst - 1))
                    nc.any.tensor_copy(
                        hoT[:, h * T + I * SB:h * T + (I + 1) * SB], oT[:])

        # ---------------- Phase 3: o_proj partial --------------------------
        if stage >= 3:
         with tc.tile_pool(name="ob", bufs=4) as ob_pool, \
             tc.tile_pool(name="psC", bufs=4, space="PSUM") as psC:
            for i in range(NT):
                for e in range(4):
                    po = psC.tile([P, SB], F32, tag="po")
                    for kk in range(2):
                        nc.tensor.matmul(
                            po[:],
                            mm(hoT[:, kk * T + i * P:kk * T + (i + 1) * P]),
                            mm(wo[:, kk * C + e * SB:kk * C + (e + 1) * SB]),
                            start=(kk == 0), stop=(kk == 1))
                    ob = ob_pool.tile([P, SB], F32, tag="ob")
                    nc.any.tensor_copy(ob[:], po[:])
                    nc.sync.dma_start(out_d[i * P:(i + 1) * P,
                                            e * SB:(e + 1) * SB], ob[:])

        if stage < 3:
            nc.sync.dma_start(out_d[0:P, :].bitcast(mm_dt), kT[:])
            nc.sync.dma_start(out_d[P:2*P, :].bitcast(mm_dt), vv[:])
            if stage >= 2:
                for a in range(2):
                    nc.sync.dma_start(out_d[(2+a)*P:(3+a)*P, :].bitcast(mm_dt),
                                      hoT[:, a*T:(a+1)*T])

    nc.compile()
    return nc


def _host_inputs(x, Wq, Wk, Wv, Wo, gq, gk, temp):
    """Build the 8 per-core input maps."""
    x2 = np.ascontiguousarray(x.reshape(T, C), dtype=np.float32)
    scale = np.float32(min(np.exp(np.float32(temp)), np.float32(50.0)))
    fk = (gq.astype(np.float32) * gk.astype(np.float32) * scale).reshape(P, 1)
    fk = np.ascontiguousarray(fk)
    ident = np.eye(P, dtype=np.float32)
    # masks: [128, 5*512]; block p: 0 where col <= 128p + row else -1e30;
    # block 4: zeros (non-diagonal chunks)
    masks = np.zeros((P, 5 * SB), dtype=np.float32)
    r = np.arange(P)[:, None]
    c = np.arange(SB)[None, :]
    for p in range(4):
        masks[:, p * SB:(p + 1) * SB] = np.where(c <= P * p + r, 0.0, NEG_BIG)
    in_maps = []
    for core in range(N_CORES):
        q0 = core * 256
        kv0 = (core // 2) * P
        wqkv = np.concatenate([Wq[:, q0:q0 + 256],
                               Wk[:, kv0:kv0 + P],
                               Wv[:, kv0:kv0 + P]], axis=1)
        in_maps.append({
            "x": x2,
            "wqkv": np.ascontiguousarray(wqkv, dtype=np.float32),
            "wo": np.ascontiguousarray(Wo[q0:q0 + 256, :], dtype=np.float32),
            "fk": fk,
            "masks": masks,
            "ident": ident,
        })
    return in_maps


_NC_CACHE = {}


def _get_nc(mm_dt=F32R, tr_dt=F32):
    key = (mm_dt, tr_dt)
    if key not in _NC_CACHE:
        _NC_CACHE[key] = _build(mm_dt, tr_dt)
    return _NC_CACHE[key]


def run(inputs, trace=False):
    nc = _get_nc()
    in_maps = _host_inputs(**inputs)
    res = bass_utils.run_bass_kernel_spmd(
        nc, in_maps, core_ids=list(range(N_CORES)), trace=trace)
    acc = res.results[0]["out"].astype(np.float32)
    for corer in res.results[1:]:
        acc = acc + corer["out"]
    return acc.reshape(1, T, C), res


def kernel(**inputs):
    out, _ = run(inputs, trace=False)
    return out



# revision 31
# speedup vs baseline: 1.9576x; 1.9576x over previous
"""GroupedQueryAttention (B=1, T=2048, C=2048, H=16, KVH=4, D=128) on 8 trn2 cores.

Sharding: tensor-parallel over heads. Core c owns q-heads {2c, 2c+1} and kv-head
c//2. Wq/Wk/Wv column-sliced, Wo row-sliced on the host; each core computes a
full o_proj partial [2048, 2048] (bf16) and the host sums the 8 partials.

Per-core pipeline:
  1. x^T is built on the HOST (free) and streamed as lhsT tiles; no PE
     transposes for x.
  2. qkv = x @ [Wq_c | Wk_c | Wv_c] in f32r (full-rate fp32).
  3. per-head LayerNorm in [t, d] layout: bn_stats/bn_aggr on Vector,
     batched Sqrt on Scalar, apply via ACT Identity (bias/scale per
     partition). softmax scale * gq * gk folded into k^T rows.
  4. PE-transpose q,k -> q^T [d, t], k^T [d, s] (f32r); v kept as [s, d] bf16.
  5. causal attention per head, t-superblocks of 512:
     S = qT.T @ kT chunks (f32r, diagonal superblocks width-trimmed),
     fused mask+running-max via tensor_tensor_reduce (Vector),
     single Exp pass with fused row-sum (accum_out), output bf16 in-place
     over S, P^T via PE-transpose against diag(1/rowsum) -> normalized
     P^T in bf16, attn@v in bf16 -> headout^T [d, t].
  6. o_proj partial per superblock: hoT(bf16) @ Wo(bf16) -> bf16 DRAM.
"""

from contextlib import ExitStack

import numpy as np

import concourse.bass as bass
import concourse.bacc as bacc
import concourse.tile as tile
from concourse import mybir
from concourse import bass_utils

P = 128
T = 2048
C = 2048
NT = T // P  # 16 t-tiles
SB = 512
NSB = T // SB  # 4 superblocks
F32 = mybir.dt.float32
F32R = mybir.dt.float32r
BF16 = mybir.dt.bfloat16
AF = mybir.ActivationFunctionType
ALU = mybir.AluOpType
AX = mybir.AxisListType
NEG_BIG = -1.0e30

N_CORES = 8
USE_TTR = False


def _build():
    nc = bacc.Bacc("TRN2", target_bir_lowering=False, debug=False,
                   num_devices=N_CORES)
    xt_d = nc.dram_tensor("xt", [NT, P, T], F32, kind="ExternalInput").ap()
    wqkv_d = nc.dram_tensor("wqkv", [C, 512], F32, kind="ExternalInput").ap()
    wo_d = nc.dram_tensor("wo", [2, P, C], F32, kind="ExternalInput").ap()
    fk_d = nc.dram_tensor("fk", [P, 1], F32, kind="ExternalInput").ap()
    masks_d = nc.dram_tensor("masks", [P, 5 * SB], F32, kind="ExternalInput").ap()
    ident_d = nc.dram_tensor("ident", [P, P], F32, kind="ExternalInput").ap()
    out_d = nc.dram_tensor("out", [T, C], F32, kind="ExternalOutput").ap()

    with tile.TileContext(nc) as tc, ExitStack() as ctx:
        const = ctx.enter_context(tc.tile_pool(name="const", bufs=1))
        persist = ctx.enter_context(tc.tile_pool(name="persist", bufs=1))
        stats = ctx.enter_context(tc.tile_pool(name="stats", bufs=32))
        xt_p = ctx.enter_context(tc.tile_pool(name="xt", bufs=2))
        qln_p = ctx.enter_context(tc.tile_pool(name="qln", bufs=2))
        s_pool = ctx.enter_context(tc.tile_pool(name="sbuf_s", bufs=1))
        p_pool = ctx.enter_context(tc.tile_pool(name="sbuf_p", bufs=2))
        dg_p = ctx.enter_context(tc.tile_pool(name="dg", bufs=8))
        pt_pool = ctx.enter_context(tc.tile_pool(name="pts", bufs=3))
        ob_pool = ctx.enter_context(tc.tile_pool(name="ob", bufs=4))
        psAcc = ctx.enter_context(tc.tile_pool(name="psAcc", bufs=2, space="PSUM"))
        psT = ctx.enter_context(tc.tile_pool(name="psT", bufs=1, space="PSUM"))
        psS = ctx.enter_context(tc.tile_pool(name="psS", bufs=2, space="PSUM"))
        psPT = ctx.enter_context(tc.tile_pool(name="psPT", bufs=2, space="PSUM"))
        psO = ctx.enter_context(tc.tile_pool(name="psO", bufs=1, space="PSUM"))

        ident = const.tile([P, P], F32, tag="ident")
        nc.sync.dma_start(ident[:], ident_d)
        fk = const.tile([P, 1], F32, tag="fk")
        nc.sync.dma_start(fk[:], fk_d)
        masks = const.tile([P, 5 * SB], F32, tag="masks")
        nc.sync.dma_start(masks[:], masks_d)
        wo = const.tile([P, 2 * C], BF16, tag="wo")
        for kk in range(2):
            wstg = xt_p.tile([P, C], F32R, tag="xT")
            nc.sync.dma_start(wstg[:], wo_d[kk].bitcast(F32R))
            nc.vector.tensor_copy(wo[:, kk * C:(kk + 1) * C],
                                  wstg[:].bitcast(F32))
        wq = persist.tile([P, NT * 512], F32R, tag="wqkv")
        for k in range(NT):
            nc.sync.dma_start(wq[:, k * 512:(k + 1) * 512],
                              wqkv_d[k * P:(k + 1) * P, :].bitcast(F32R))

        qT = persist.tile([P, 2 * T], F32R, tag="qT")   # [d, t] per head
        kT = persist.tile([P, T], F32R, tag="kT")       # [d, s]
        vv = persist.tile([P, T], BF16, tag="vv")       # [s, d], s-tile j at j*128
        hoT = persist.tile([P, 2 * T], BF16, tag="hoT")  # [d, t] per head

        epsb = const.tile([P, 1], F32, tag="epsb")
        nc.vector.memset(epsb[:], 1e-5)
        identf = ident[:]

        # ---------------- Phase 1: projections + LN + transposes ----------
        for i in range(NT):
            xTt = xt_p.tile([P, C], F32R, tag="xT")
            for k in range(NT):
                nc.sync.dma_start(xTt[:, k * P:(k + 1) * P],
                                  xt_d[k][:, i * P:(i + 1) * P].bitcast(F32R))
            qk = psAcc.tile([P, 512], F32, tag="acc")
            for k in range(NT):
                nc.tensor.matmul(qk[:], xTt[:, k * P:(k + 1) * P],
                                 wq[:, k * 512:(k + 1) * 512],
                                 start=(k == 0), stop=(k == NT - 1))
            # LayerNorm stats for q0, q1, k (cols 0:128, 128:256, 256:384)
            st6 = stats.tile([P, 18], F32, tag="st6")
            mv = stats.tile([P, 6], F32, tag="mv")
            sd = stats.tile([P, 3], F32, tag="sd")
            rs = stats.tile([P, 3], F32, tag="rs")
            nb = stats.tile([P, 3], F32, tag="nb")
            for ci in range(3):
                nc.vector.bn_stats(st6[:, ci * 6:(ci + 1) * 6],
                                   qk[:, ci * P:(ci + 1) * P])
                nc.vector.bn_aggr(mv[:, ci * 2:ci * 2 + 2],
                                  st6[:, ci * 6:(ci + 1) * 6])
            for ci in range(3):
                nc.scalar.activation(sd[:, ci:ci + 1], mv[:, ci * 2 + 1:ci * 2 + 2],
                                     AF.Sqrt, bias=epsb[:])
            nc.vector.reciprocal(rs[:], sd[:])
            for ci in range(3):
                nc.vector.scalar_tensor_tensor(
                    nb[:, ci:ci + 1], mv[:, ci * 2:ci * 2 + 1], -1.0,
                    rs[:, ci:ci + 1], ALU.mult, ALU.mult)
            qln = qln_p.tile([P, 384], F32, tag="qln")
            for ci in range(3):
                nc.scalar.activation(qln[:, ci * P:(ci + 1) * P],
                                     qk[:, ci * P:(ci + 1) * P],
                                     AF.Identity, bias=nb[:, ci:ci + 1],
                                     scale=rs[:, ci:ci + 1])
            pt = psT.tile([P, 384], F32, tag="tp")
            for ci in range(3):
                nc.tensor.transpose(pt[:, ci * P:(ci + 1) * P],
                                    qln[:, ci * P:(ci + 1) * P],
                                    identf)
            for h in range(2):
                nc.vector.tensor_copy(
                    qT[:, h * T + i * P:h * T + (i + 1) * P],
                    pt[:, h * P:(h + 1) * P])
            nc.vector.tensor_scalar_mul(kT[:, i * P:(i + 1) * P],
                                        pt[:, 2 * P:3 * P], fk[:])
            nc.vector.tensor_copy(vv[:, i * P:(i + 1) * P], qk[:, 384:512])

        # ---------------- Phase 2: causal attention + o_proj --------------
        for I in range(NSB):
            for h in range(2):
                S = s_pool.tile([P, 4 * T], F32, tag="S")
                Pb = p_pool.tile([P, 4 * T], BF16, tag="P")
                m = [None] * 4
                for J in range(I + 1):
                    diag = (J == I)
                    for p in range(4):
                        W = (p + 1) * P if diag else SB
                        sp = psS.tile([P, SB], F32, tag="sp")
                        lq = qT[:, h * T + (I * 4 + p) * P:
                                h * T + (I * 4 + p + 1) * P]
                        nc.tensor.matmul(sp[:, 0:W], lq,
                                         kT[:, J * SB:J * SB + W],
                                         start=True, stop=True)
                        msl = (masks[:, p * SB:p * SB + W] if diag
                               else masks[:, 4 * SB:4 * SB + W])
                        mnew = stats.tile([P, 1], F32, tag="st")
                        if USE_TTR:
                            nc.vector.tensor_tensor_reduce(
                                out=S[:, p * T + J * SB:p * T + J * SB + W],
                                in0=sp[:, 0:W], in1=msl, scale=1.0,
                                scalar=(m[p][:] if m[p] is not None else -3.0e38),
                                op0=ALU.add, op1=ALU.max, accum_out=mnew[:])
                        else:
                            dst = S[:, p * T + J * SB:p * T + J * SB + W]
                            nc.vector.scalar_tensor_tensor(
                                dst, sp[:, 0:W], 1.0, msl, ALU.mult, ALU.add)
                            if m[p] is None:
                                nc.vector.reduce_max(mnew[:], dst, AX.X)
                            else:
                                cmax = stats.tile([P, 1], F32, tag="st")
                                nc.vector.reduce_max(cmax[:], dst, AX.X)
                                nc.vector.scalar_tensor_tensor(
                                    mnew[:], cmax[:], 1.0, m[p][:],
                                    ALU.mult, ALU.max)
                        m[p] = mnew
                D = [None] * 4
                for p in range(4):
                    L = I * SB + (p + 1) * P
                    nm = stats.tile([P, 1], F32, tag="st")
                    nc.vector.tensor_scalar_mul(nm[:], m[p][:], -1.0)
                    sm = stats.tile([P, 1], F32, tag="st")
                    nc.scalar.activation(Pb[:, p * T:p * T + L],
                                         S[:, p * T:p * T + L],
                                         AF.Exp, bias=nm[:], accum_out=sm[:])
                    rs1 = stats.tile([P, 1], F32, tag="st")
                    nc.vector.reciprocal(rs1[:], sm[:])
                    Dp = dg_p.tile([P, P], BF16, tag="D")
                    nc.vector.tensor_scalar_mul(Dp[:], ident[:], rs1[:])
                    D[p] = Dp
                oT = psO.tile([P, SB], F32, tag="oT")
                nst = 4 * (I + 1)
                for j in range(nst):
                    jl = j - 4 * I
                    p0 = jl if jl > 0 else 0
                    c0 = p0 * P
                    ptp = psPT.tile([P, SB], F32, tag="ptp")
                    for p in range(p0, 4):
                        # P_block.T @ diag(1/rowsum): transposed AND
                        # normalized in one PE pass (regular matmul;
                        # transpose-mode would ignore the diag values).
                        nc.tensor.matmul(
                            ptp[:, p * P:(p + 1) * P],
                            Pb[:, p * T + j * P:p * T + (j + 1) * P],
                            D[p][:], start=True, stop=True)
                    pts = pt_pool.tile([P, SB], BF16, tag="pts")
                    nc.scalar.copy(pts[:, c0:SB], ptp[:, c0:SB])
                    nc.tensor.matmul(oT[:, c0:SB], vv[:, j * P:(j + 1) * P],
                                     pts[:, c0:SB],
                                     start=(j == 0), stop=(j == nst - 1))
                nc.vector.tensor_copy(
                    hoT[:, h * T + I * SB:h * T + (I + 1) * SB], oT[:])
            # o_proj for the 4 t-tiles of superblock I (needs both heads)
            for i in range(4 * I, 4 * I + 4):
                for e in range(4):
                    po = psAcc.tile([P, SB], F32, tag="acc")
                    for kk in range(2):
                        nc.tensor.matmul(
                            po[:],
                            hoT[:, kk * T + i * P:kk * T + (i + 1) * P],
                            wo[:, kk * C + e * SB:kk * C + (e + 1) * SB],
                            start=(kk == 0), stop=(kk == 1))
                    ob = ob_pool.tile([P, SB], F32, tag="ob")
                    nc.vector.tensor_copy(ob[:], po[:])
                    nc.sync.dma_start(out_d[i * P:(i + 1) * P,
                                            e * SB:(e + 1) * SB], ob[:])

    nc.compile()
    return nc


def _host_inputs(x, Wq, Wk, Wv, Wo, gq, gk, temp):
    """Build the 8 per-core input maps."""
    bf16 = mybir.dt.np(BF16)
    x2 = np.asarray(x, dtype=np.float32).reshape(T, C)
    xt = np.ascontiguousarray(x2.T).reshape(NT, P, T)
    scale = np.float32(min(np.exp(np.float32(temp)), np.float32(50.0)))
    fk = (np.asarray(gq, np.float32) * np.asarray(gk, np.float32)
          * scale).reshape(P, 1)
    fk = np.ascontiguousarray(fk)
    ident = np.eye(P, dtype=np.float32)
    # masks: [128, 5*512]; block p: 0 where col <= 128p + row else -1e30;
    # block 4: zeros (non-diagonal chunks)
    masks = np.zeros((P, 5 * SB), dtype=np.float32)
    r = np.arange(P)[:, None]
    c = np.arange(SB)[None, :]
    for p in range(4):
        masks[:, p * SB:(p + 1) * SB] = np.where(c <= P * p + r, 0.0, NEG_BIG)
    Wq = np.asarray(Wq, np.float32)
    Wk = np.asarray(Wk, np.float32)
    Wv = np.asarray(Wv, np.float32)
    Wo = np.asarray(Wo, np.float32)
    in_maps = []
    for core in range(N_CORES):
        q0 = core * 256
        kv0 = (core // 2) * P
        wqkv = np.concatenate([Wq[:, q0:q0 + 256],
                               Wk[:, kv0:kv0 + P],
                               Wv[:, kv0:kv0 + P]], axis=1)
        in_maps.append({
            "xt": xt,
            "wqkv": np.ascontiguousarray(wqkv, dtype=np.float32),
            "wo": np.ascontiguousarray(Wo[q0:q0 + 256, :]).reshape(2, P, C),
            "fk": fk,
            "masks": masks,
            "ident": ident,
        })
    return in_maps


_NC_CACHE = {}


def _get_nc():
    if "nc" not in _NC_CACHE:
        _NC_CACHE["nc"] = _build()
    return _NC_CACHE["nc"]


def run(inputs, trace=False):
    nc = _get_nc()
    in_maps = _host_inputs(**inputs)
    res = bass_utils.run_bass_kernel_spmd(
        nc, in_maps, core_ids=list(range(N_CORES)), trace=trace)
    acc = res.results[0]["out"].astype(np.float32)
    for corer in res.results[1:]:
        acc = acc + corer["out"].astype(np.float32)
    return acc.reshape(1, T, C), res


def kernel(**inputs):
    out, _ = run(inputs, trace=False)
    return out
